# revision 46
# baseline (speedup 1.0000x reference)
"""Trainium2 Bass kernel for the BettingLoss problem.

Strategy (pure data parallel, 8 NeuronCores):
  - Shard the batch dim of the three [B, 6] f32 tensors into 8 contiguous
    row blocks, one per core, laid out trap-major [128, 6, 4096] so every
    on-chip per-trap slice is dense.
  - Host packs two fp16 tensors per core (probs_s, odds_s) with the win
    flag folded into the SIGN of both (their product stays positive) and
    alpha=1.1 folded into odds_s.  This removes the separate winners
    tensor and halves the remaining bytes: 12.6 MB/core instead of 28.3.
  - Per row on device (all fp16, DVE 2x/4x perf modes):
        raw_t = probs_s_t * odds_s_t          (= odds*1.1*probs >= 0)
        best  = max_t raw_t                   (3-op pairwise tree)
        b1    = max(best, 1 + 2^-10)          (fp16 "just above 1")
        sel_t = raw_t >= b1                   (argmax trap(s), bet rows only)
        z_t   = sel_t * odds_s_t              (signed odds at argmax)
    ScalarE accumulates per-partition partials (last tile on DVE via
    scalar_tensor_tensor to shorten the pipeline tail):
        SWO  = sum relu(-z)        = sum bet * win * odds * 1.1
        RELU = sum relu(best - 1)  -> sum_bet_ep / 0.019
        NBS  = sum sign(best - 1)  -> num_bets = (NBS + rows) / 2
  - Host combines partials in float64:
        loss         = -(0.019 * RELU) / B    (fallback if num_bets == 0)
        batch_profit = 0.019 * SWO - 0.019 * num_bets
"""

import sys

if "/opt/trn_rl_repo" not in sys.path:
    sys.path.insert(0, "/opt/trn_rl_repo")

import numpy as np

B = 4_194_304
T = 6
N_CORES = 8
BC = B // N_CORES          # rows per core
P = 128                    # SBUF partitions
ROWS_PP = BC // P          # rows per partition per core (4096)
R = 896                    # max rows per partition per tile
# ~1.4x ramp keeps DVE fed during the DMA-limited start (DVE consumes rows
# only ~1.31x slower than DMA delivers them, so tile growth must stay
# under that ratio); a small tail tile keeps the final reduction short.
ROW_TILES = [128, 224, 352, 512, 768, 896, 896, 320]
NTT = len(ROW_TILES)       # tiles per core
F = T * R                  # free-dim elements per input tile

ALPHA = 1.1
COMMISSION = 0.05
BET_PCT = 0.02
PAYOUT_SCALE = BET_PCT * (1.0 - COMMISSION)          # 0.019
ONE_PLUS = 1.0009765625    # smallest fp16 strictly greater than 1.0

_PROGRAM = None


def _build_program():
    from concourse import bacc, mybir
    from concourse.tile import TileContext

    DT16 = mybir.dt.float16
    DT32 = mybir.dt.float32
    Alu = mybir.AluOpType
    Act = mybir.ActivationFunctionType

    nc = bacc.Bacc("TRN2", target_bir_lowering=False, debug=False,
                   num_devices=N_CORES)
    # Tile-contiguous layout: tile k's [2, T, rr_k] block is a single dense
    # run per partition, so each tile DMA is one descriptor per partition
    # instead of 12 short strided chunks.
    po_d = nc.dram_tensor("po", [P, 2 * T * ROWS_PP], DT16,
                          kind="ExternalInput").ap()
    acc_d = nc.dram_tensor("acc", [P, 3 * NTT], DT32,
                           kind="ExternalOutput").ap()

    with TileContext(nc) as tc:
        with tc.tile_pool(name="io", bufs=5) as io_pool, \
             tc.tile_pool(name="iotail", bufs=1) as iot_pool, \
             tc.tile_pool(name="selp", bufs=3) as sel_pool, \
             tc.tile_pool(name="small", bufs=5) as sm_pool, \
             tc.tile_pool(name="accp", bufs=1) as acc_pool:
            # cols [0:NTT) relu, [NTT:2NTT) nb, [2NTT:3NTT) swo
            acc = acc_pool.tile([P, 3 * NTT], DT32)
            acc2 = acc_pool.tile([P, 3 * NTT], DT32)
            neg1 = acc_pool.tile([P, 1], DT32)
            nc.vector.memset(neg1[:], -1.0)
            # in1 operand for the last tile's stt stats; memset runs on
            # GpSimd so it never delays the DVE pipeline start
            zeros6 = acc_pool.tile([P, T * ROW_TILES[-1]], DT16)
            nc.gpsimd.memset(zeros6[:], 0.0)
            assert sum(ROW_TILES) == ROWS_PP
            # the last two tiles arrive in ONE transfer (their host blocks
            # are adjacent), saving a per-transfer ring overhead on the
            # latest-needed data; compute still runs per sub-tile
            TAILR = ROW_TILES[-2] + ROW_TILES[-1]
            potB = iot_pool.tile([P, 2 * T * TAILR], DT16)
            r0 = 0
            for k, rr in enumerate(ROW_TILES):
                fr = T * rr
                last = k == NTT - 1
                if k == NTT - 2:
                    nc.sync.dma_start(
                        out=potB[:], in_=po_d[:, 2 * T * r0:
                                              2 * T * (r0 + TAILR)])
                    pot = potB[:, :2 * fr]
                elif last:
                    pot = potB[:, 2 * T * ROW_TILES[-2]:]
                else:
                    pot = io_pool.tile([P, 2 * F], DT16, tag="pot",
                                       name=f"pot{k}")[:, :2 * fr]
                    nc.sync.dma_start(out=pot, in_=po_d[:, 2 * T * r0:
                                                        2 * T * (r0 + rr)])
                r0 += rr

                po4 = pot.rearrange("p (c t n) -> p c t n", c=2, t=T)
                pt = po4[:, 0]          # probs_s -> raw (in place)
                ot = po4[:, 1]          # odds_s

                # sel buffer doubles as m3 scratch (pair maxes) before the
                # is_ge overwrites it, then holds z in place after the mult
                sel = sel_pool.tile([P, F], DT16, tag="sel",
                                    name=f"sel{k}")[:, :fr]
                sel3 = sel.rearrange("p (t n) -> p t n", t=T)
                m33 = sel3[:, 0:3, :]
                best = sm_pool.tile([P, R], DT16, tag="best",
                                    name=f"best{k}")[:, :rr]
                b1 = sm_pool.tile([P, R], DT16, tag="b1",
                                  name=f"b1{k}")[:, :rr]
                junk = sm_pool.tile([P, R], DT16, tag="junk",
                                    name=f"junk{k}")[:, :rr]

                # raw = probs_s * odds_s  (>= 0; win is sign-encoded in both)
                nc.vector.tensor_tensor(pt, pt, ot, op=Alu.mult)
                # best = max over 6 traps: pairwise tree
                nc.vector.tensor_tensor(m33, pt[:, 0:3, :], pt[:, 3:6, :],
                                        op=Alu.max)
                nc.vector.tensor_tensor(best, m33[:, 0, :], m33[:, 1, :],
                                        op=Alu.max)
                nc.vector.tensor_tensor(best, best, m33[:, 2, :], op=Alu.max)
                # b1 = max(best, 1+ulp): > 1 threshold, rows w/o bet select 0
                nc.vector.tensor_scalar(b1, best, float(ONE_PLUS), None,
                                        op0=Alu.max)
                # sel_t = raw_t >= b1 (first/any argmax on bet rows)
                nc.vector.tensor_tensor(
                    sel3, pt, b1.unsqueeze(1).broadcast_to([P, T, rr]),
                    op=Alu.is_ge)
                # z = sel * odds_s in place (io tile released by DVE itself,
                # so the next DMA is not gated on ScalarE)
                nc.vector.tensor_tensor(sel3, sel3, ot, op=Alu.mult)

                # Stats. RELU = sum relu(best-1); NBS = sum sign(best-1)
                # -> num_bets = (NBS + B)/2; SWO = sum relu(-z).
                # ScalarE handles all but the last tile (RELU/SIGN depend
                # only on best, so they overlap the DVE sel/z of the same
                # tile; the big SWO reduction reads z last).  The last
                # tile's stats run on DVE via scalar_tensor_tensor while
                # ScalarE drains the previous tile, shortening the tail.
                # (Last tile's nb column holds a 0/1 count, not a sign sum.)
                if not last:
                    nc.scalar.activation(junk, best, Act.Relu,
                                         bias=neg1[:], scale=1.0,
                                         accum_out=acc[:, k:k + 1])
                    nc.scalar.activation(junk, best, Act.Sign,
                                         bias=neg1[:], scale=1.0,
                                         accum_out=acc[:, NTT + k:NTT + k + 1])
                    nc.scalar.activation(sel, sel, Act.Relu, scale=-1.0,
                                         accum_out=acc[:, 2 * NTT + k:2 * NTT + k + 1])
                else:
                    # SWO on ScalarE (idle by now; its accumulate-read runs
                    # in parallel with DVE's stt reads), best-stats on DVE
                    nc.scalar.activation(sel, sel, Act.Relu, scale=-1.0,
                                         accum_out=acc[:, 2 * NTT + k:2 * NTT + k + 1])
                    nc.vector.scalar_tensor_tensor(
                        junk, best, -1.0, zeros6[:, :rr],
                        op0=Alu.add, op1=Alu.max,
                        accum_out=acc[:, k:k + 1])
                    nc.vector.scalar_tensor_tensor(
                        junk, best, 1.0, zeros6[:, :rr],
                        op0=Alu.is_gt, op1=Alu.max,
                        accum_out=acc[:, NTT + k:NTT + k + 1])

            # Copy through a shadow tile on DVE before the writeback: Tile
            # does not track scalar_tensor_tensor accum_out as a write to
            # acc, so a direct DMA of acc races the last stt partials (the
            # copy is ordered after them by DVE program order, and after
            # the ScalarE accumulate-reads by the tracked read deps).
            nc.vector.tensor_copy(acc2[:], acc[:])
            nc.sync.dma_start(out=acc_d, in_=acc2[:])

    nc.compile()
    return nc


def _get_program():
    global _PROGRAM
    if _PROGRAM is None:
        _PROGRAM = _build_program()
    return _PROGRAM


def _shard_po(probs, win, odds, i):
    """Core i's [BC, 6] rows as fp16 [P, 2*T*ROWS_PP], tile-contiguous.

    Win folded into the sign of both probs and odds (product stays
    positive), alpha folded into odds.  Within each row tile the block is
    [2, T, rr] trap-major so per-trap on-chip slices are dense and each
    tile's DMA is one contiguous run per partition.
    """
    s = slice(i * BC, (i + 1) * BC)
    sgn = 1.0 - 2.0 * win[s]
    ps = (probs[s] * sgn).astype(np.float16) \
        .reshape(P, ROWS_PP, T).transpose(0, 2, 1)      # [P, T, ROWS_PP]
    os_ = (odds[s] * (np.float32(ALPHA) * sgn)).astype(np.float16) \
        .reshape(P, ROWS_PP, T).transpose(0, 2, 1)
    out = np.empty((P, 2 * T * ROWS_PP), np.float16)
    off = 0
    r0 = 0
    for rr in ROW_TILES:
        blk = 2 * T * rr
        out[:, off:off + blk] = np.concatenate(
            (ps[:, :, r0:r0 + rr], os_[:, :, r0:r0 + rr]), axis=1
        ).reshape(P, blk)
        off += blk
        r0 += rr
    return out


def _install_ntff_shim():
    """Provide antenv.axon_hooks (missing in this image) so trace=True works.

    Replicates trn_agent_boot's ctypes NTFF hook against libaxon_pjrt.so.
    Only used for profiling runs; plain kernel() calls never need it.
    """
    import contextlib
    import ctypes
    import types

    if "antenv.axon_hooks" in sys.modules:
        return
    try:
        from antenv import axon_hooks  # noqa: F401
        return
    except ImportError:
        pass

    so_path = "/opt/axon/libaxon_pjrt.so"
    hook = None
    try:
        lib = ctypes.CDLL(so_path)
        if hasattr(lib, "axon_start_nrt_profile"):
            lib.axon_start_nrt_profile.argtypes = [
                ctypes.POINTER(ctypes.c_int64), ctypes.c_size_t]
            lib.axon_start_nrt_profile.restype = ctypes.c_int64
            lib.axon_stop_nrt_profile.argtypes = [ctypes.c_char_p]
            lib.axon_stop_nrt_profile.restype = ctypes.c_int64

            @contextlib.contextmanager
            def _hook(output_dir, device_ids):
                import jax
                jax.devices()
                if device_ids:
                    ids = (ctypes.c_int64 * len(device_ids))(*device_ids)
                    rc = lib.axon_start_nrt_profile(ids, len(device_ids))
                else:
                    rc = lib.axon_start_nrt_profile(None, 0)
                if rc != 0:
                    raise RuntimeError(f"axon_start_nrt_profile rc={rc}")
                try:
                    yield
                finally:
                    n = lib.axon_stop_nrt_profile(str(output_dir).encode())
                    print(f"profile: {n} file(s) written to {output_dir}",
                          file=sys.stderr)

            hook = _hook
    except OSError:
        pass

    mod = types.ModuleType("antenv.axon_hooks")
    mod.get_axon_ntff_profile_hook = lambda: hook
    mod.set_axon_ntff_profile_hook = lambda h: None
    sys.modules["antenv.axon_hooks"] = mod


def _run_device(predicted_probs, true_winners, market_odds, trace=False):
    from concourse.bass_utils import run_bass_kernel_spmd

    if trace:
        _install_ntff_shim()
    nc = _get_program()
    in_maps = []
    for i in range(N_CORES):
        in_maps.append({
            "po": _shard_po(predicted_probs, true_winners, market_odds, i),
        })
    res = run_bass_kernel_spmd(nc, in_maps, list(range(N_CORES)), trace=trace)
    return res


def kernel(predicted_probs, true_winners, market_odds, _trace=False,
           _result_holder=None):
    res = _run_device(predicted_probs, true_winners, market_odds, trace=_trace)
    if _result_holder is not None:
        _result_holder.append(res)

    RELU = 0.0
    S_WO = 0.0
    NBS = 0.0
    NB_LAST = 0.0
    for i in range(N_CORES):
        a_s = res.results[i]["acc"].astype(np.float64)
        RELU += a_s[:, :NTT].sum()
        NBS += a_s[:, NTT:2 * NTT - 1].sum()
        NB_LAST += a_s[:, 2 * NTT - 1].sum()
        S_WO += a_s[:, 2 * NTT:].sum()
    # NBS = sum sign(best-1) over all but the last tile; the last tile
    # reports a direct count of best > 1 rows.
    B_REST = (ROWS_PP - ROW_TILES[-1]) * P * N_CORES
    num_bets = int(round((NBS + B_REST) / 2 + NB_LAST))

    if num_bets > 0:
        total_expected_profit = PAYOUT_SCALE * RELU
    else:
        total_expected_profit = -np.float64(
            np.mean(np.max(predicted_probs, axis=1))) * 0.1
    loss = -total_expected_profit / B
    batch_profit = PAYOUT_SCALE * S_WO - PAYOUT_SCALE * num_bets

    return (np.float32(loss), np.float32(batch_profit), np.int32(num_bets))


if __name__ == "__main__":
    rng = np.random.default_rng(0)
    probs = rng.random((B, T), dtype=np.float32)
    win = (rng.random((B, T)) > 0.8).astype(np.float32)
    odds = rng.random((B, T), dtype=np.float32) * 10.0
    odds[rng.random((B, 1))[:, 0] < 0.1] = 0.0
    out = kernel(probs, win, odds)
    print("kernel out:", out)


# revision 47
# speedup vs baseline: 1.2116x; 1.2116x over previous
"""Trainium2 Bass kernel for the BettingLoss problem.

Strategy (pure data parallel, 8 NeuronCores):
  - Shard the batch dim of the three [B, 6] f32 tensors into 8 contiguous
    row blocks, one per core, laid out trap-major [128, 6, 4096] so every
    on-chip per-trap slice is dense.
  - Host packs two fp16 tensors per core (probs_s, odds_s) with the win
    flag folded into the SIGN of both (their product stays positive) and
    alpha=1.1 folded into odds_s.  This removes the separate winners
    tensor and halves the remaining bytes: 12.6 MB/core instead of 28.3.
  - Per row on device (all fp16, DVE 2x/4x perf modes):
        raw_t = probs_s_t * odds_s_t          (= odds*1.1*probs >= 0)
        best  = max_t raw_t                   (3-op pairwise tree)
        b1    = max(best, 1 + 2^-10)          (fp16 "just above 1")
        sel_t = raw_t >= b1                   (argmax trap(s), bet rows only)
        z_t   = sel_t * odds_s_t              (signed odds at argmax)
    ScalarE accumulates per-partition partials (last tile on DVE via
    scalar_tensor_tensor to shorten the pipeline tail):
        SWO  = sum relu(-z)        = sum bet * win * odds * 1.1
        RELU = sum relu(best - 1)  -> sum_bet_ep / 0.019
        NBS  = sum sign(best - 1)  -> num_bets = (NBS + rows) / 2
  - Host combines partials in float64:
        loss         = -(0.019 * RELU) / B    (fallback if num_bets == 0)
        batch_profit = 0.019 * SWO - 0.019 * num_bets
"""

import sys

if "/opt/trn_rl_repo" not in sys.path:
    sys.path.insert(0, "/opt/trn_rl_repo")

import numpy as np

B = 4_194_304
T = 6
N_CORES = 8
BC = B // N_CORES          # rows per core
P = 128                    # SBUF partitions
ROWS_PP = BC // P          # rows per partition per core (4096)
R = 896                    # max rows per partition per tile
# ~1.4x ramp keeps DVE fed during the DMA-limited start (DVE consumes rows
# only ~1.31x slower than DMA delivers them, so tile growth must stay
# under that ratio); a small tail tile keeps the final reduction short.
ROW_TILES = [128, 224, 352, 512, 768, 896, 896, 320]
NTT = len(ROW_TILES)       # tiles per core
F = T * R                  # free-dim elements per input tile

ALPHA = 1.1
COMMISSION = 0.05
BET_PCT = 0.02
PAYOUT_SCALE = BET_PCT * (1.0 - COMMISSION)          # 0.019
ONE_PLUS = 1.0009765625    # smallest fp16 strictly greater than 1.0

_PROGRAM = None


def _build_program():
    from concourse import bacc, mybir
    from concourse.tile import TileContext

    DT16 = mybir.dt.float16
    DT32 = mybir.dt.float32
    Alu = mybir.AluOpType
    Act = mybir.ActivationFunctionType

    nc = bacc.Bacc("TRN2", target_bir_lowering=False, debug=False,
                   num_devices=N_CORES)
    # Tile-contiguous layout: tile k's [2, T, rr_k] block is a single dense
    # run per partition, so each tile DMA is one descriptor per partition
    # instead of 12 short strided chunks.
    po_d = nc.dram_tensor("po", [P, 2 * T * ROWS_PP], DT16,
                          kind="ExternalInput").ap()
    acc_d = nc.dram_tensor("acc", [P, 3 * NTT], DT32,
                           kind="ExternalOutput").ap()

    with TileContext(nc) as tc:
        with tc.tile_pool(name="io", bufs=5) as io_pool, \
             tc.tile_pool(name="iotail", bufs=1) as iot_pool, \
             tc.tile_pool(name="selp", bufs=3) as sel_pool, \
             tc.tile_pool(name="small", bufs=5) as sm_pool, \
             tc.tile_pool(name="accp", bufs=1) as acc_pool:
            # cols [0:NTT) relu, [NTT:2NTT) nb, [2NTT:3NTT) swo
            acc = acc_pool.tile([P, 3 * NTT], DT32)
            acc2 = acc_pool.tile([P, 3 * NTT], DT32)
            neg1 = acc_pool.tile([P, 1], DT32)
            nc.vector.memset(neg1[:], -1.0)
            # in1 operand for the last tile's stt stats; memset runs on
            # GpSimd so it never delays the DVE pipeline start
            zeros6 = acc_pool.tile([P, T * ROW_TILES[-1]], DT16)
            nc.gpsimd.memset(zeros6[:], 0.0)
            assert sum(ROW_TILES) == ROWS_PP
            # the last two tiles arrive in ONE transfer (their host blocks
            # are adjacent), saving a per-transfer ring overhead on the
            # latest-needed data; compute still runs per sub-tile
            TAILR = ROW_TILES[-2] + ROW_TILES[-1]
            potB = iot_pool.tile([P, 2 * T * TAILR], DT16)
            r0 = 0
            for k, rr in enumerate(ROW_TILES):
                fr = T * rr
                last = k == NTT - 1
                if k == NTT - 2:
                    nc.sync.dma_start(
                        out=potB[:], in_=po_d[:, 2 * T * r0:
                                              2 * T * (r0 + TAILR)])
                    pot = potB[:, :2 * fr]
                elif last:
                    pot = potB[:, 2 * T * ROW_TILES[-2]:]
                else:
                    pot = io_pool.tile([P, 2 * F], DT16, tag="pot",
                                       name=f"pot{k}")[:, :2 * fr]
                    nc.sync.dma_start(out=pot, in_=po_d[:, 2 * T * r0:
                                                        2 * T * (r0 + rr)])
                r0 += rr

                po4 = pot.rearrange("p (c t n) -> p c t n", c=2, t=T)
                pt = po4[:, 0]          # probs_s -> raw (in place)
                ot = po4[:, 1]          # odds_s

                # sel buffer doubles as m3 scratch (pair maxes) before the
                # is_ge overwrites it, then holds z in place after the mult
                sel = sel_pool.tile([P, F], DT16, tag="sel",
                                    name=f"sel{k}")[:, :fr]
                sel3 = sel.rearrange("p (t n) -> p t n", t=T)
                m33 = sel3[:, 0:3, :]
                best = sm_pool.tile([P, R], DT16, tag="best",
                                    name=f"best{k}")[:, :rr]
                b1 = sm_pool.tile([P, R], DT16, tag="b1",
                                  name=f"b1{k}")[:, :rr]
                junk = sm_pool.tile([P, R], DT16, tag="junk",
                                    name=f"junk{k}")[:, :rr]

                # raw = probs_s * odds_s  (>= 0; win is sign-encoded in both)
                nc.vector.tensor_tensor(pt, pt, ot, op=Alu.mult)
                # best = max over 6 traps: pairwise tree
                nc.vector.tensor_tensor(m33, pt[:, 0:3, :], pt[:, 3:6, :],
                                        op=Alu.max)
                nc.vector.tensor_tensor(best, m33[:, 0, :], m33[:, 1, :],
                                        op=Alu.max)
                nc.vector.tensor_tensor(best, best, m33[:, 2, :], op=Alu.max)
                # b1 = max(best, 1+ulp): > 1 threshold, rows w/o bet select 0
                nc.vector.tensor_scalar(b1, best, float(ONE_PLUS), None,
                                        op0=Alu.max)
                # sel_t = raw_t >= b1 (first/any argmax on bet rows)
                nc.vector.tensor_tensor(
                    sel3, pt, b1.unsqueeze(1).broadcast_to([P, T, rr]),
                    op=Alu.is_ge)
                # z = sel * odds_s in place (io tile released by DVE itself,
                # so the next DMA is not gated on ScalarE)
                nc.vector.tensor_tensor(sel3, sel3, ot, op=Alu.mult)

                # Stats. RELU = sum relu(best-1); NBS = sum sign(best-1)
                # -> num_bets = (NBS + B)/2; SWO = sum relu(-z).
                # ScalarE handles all but the last tile (RELU/SIGN depend
                # only on best, so they overlap the DVE sel/z of the same
                # tile; the big SWO reduction reads z last).  The last
                # tile's stats run on DVE via scalar_tensor_tensor while
                # ScalarE drains the previous tile, shortening the tail.
                # (Last tile's nb column holds a 0/1 count, not a sign sum.)
                if not last:
                    nc.scalar.activation(junk, best, Act.Relu,
                                         bias=neg1[:], scale=1.0,
                                         accum_out=acc[:, k:k + 1])
                    nc.scalar.activation(junk, best, Act.Sign,
                                         bias=neg1[:], scale=1.0,
                                         accum_out=acc[:, NTT + k:NTT + k + 1])
                    nc.scalar.activation(sel, sel, Act.Relu, scale=-1.0,
                                         accum_out=acc[:, 2 * NTT + k:2 * NTT + k + 1])
                else:
                    nc.vector.scalar_tensor_tensor(
                        sel, sel, -1.0, zeros6[:, :fr],
                        op0=Alu.mult, op1=Alu.max,
                        accum_out=acc[:, 2 * NTT + k:2 * NTT + k + 1])
                    nc.vector.scalar_tensor_tensor(
                        junk, best, -1.0, zeros6[:, :rr],
                        op0=Alu.add, op1=Alu.max,
                        accum_out=acc[:, k:k + 1])
                    nc.vector.scalar_tensor_tensor(
                        junk, best, 1.0, zeros6[:, :rr],
                        op0=Alu.is_gt, op1=Alu.max,
                        accum_out=acc[:, NTT + k:NTT + k + 1])

            # Copy through a shadow tile on DVE before the writeback: Tile
            # does not track scalar_tensor_tensor accum_out as a write to
            # acc, so a direct DMA of acc races the last stt partials (the
            # copy is ordered after them by DVE program order, and after
            # the ScalarE accumulate-reads by the tracked read deps).
            nc.vector.tensor_copy(acc2[:], acc[:])
            nc.sync.dma_start(out=acc_d, in_=acc2[:])

    nc.compile()
    return nc


def _get_program():
    global _PROGRAM
    if _PROGRAM is None:
        _PROGRAM = _build_program()
    return _PROGRAM


def _shard_po(probs, win, odds, i):
    """Core i's [BC, 6] rows as fp16 [P, 2*T*ROWS_PP], tile-contiguous.

    Win folded into the sign of both probs and odds (product stays
    positive), alpha folded into odds.  Within each row tile the block is
    [2, T, rr] trap-major so per-trap on-chip slices are dense and each
    tile's DMA is one contiguous run per partition.
    """
    s = slice(i * BC, (i + 1) * BC)
    sgn = 1.0 - 2.0 * win[s]
    ps = (probs[s] * sgn).astype(np.float16) \
        .reshape(P, ROWS_PP, T).transpose(0, 2, 1)      # [P, T, ROWS_PP]
    os_ = (odds[s] * (np.float32(ALPHA) * sgn)).astype(np.float16) \
        .reshape(P, ROWS_PP, T).transpose(0, 2, 1)
    out = np.empty((P, 2 * T * ROWS_PP), np.float16)
    off = 0
    r0 = 0
    for rr in ROW_TILES:
        blk = 2 * T * rr
        out[:, off:off + blk] = np.concatenate(
            (ps[:, :, r0:r0 + rr], os_[:, :, r0:r0 + rr]), axis=1
        ).reshape(P, blk)
        off += blk
        r0 += rr
    return out


def _install_ntff_shim():
    """Provide antenv.axon_hooks (missing in this image) so trace=True works.

    Replicates trn_agent_boot's ctypes NTFF hook against libaxon_pjrt.so.
    Only used for profiling runs; plain kernel() calls never need it.
    """
    import contextlib
    import ctypes
    import types

    if "antenv.axon_hooks" in sys.modules:
        return
    try:
        from antenv import axon_hooks  # noqa: F401
        return
    except ImportError:
        pass

    so_path = "/opt/axon/libaxon_pjrt.so"
    hook = None
    try:
        lib = ctypes.CDLL(so_path)
        if hasattr(lib, "axon_start_nrt_profile"):
            lib.axon_start_nrt_profile.argtypes = [
                ctypes.POINTER(ctypes.c_int64), ctypes.c_size_t]
            lib.axon_start_nrt_profile.restype = ctypes.c_int64
            lib.axon_stop_nrt_profile.argtypes = [ctypes.c_char_p]
            lib.axon_stop_nrt_profile.restype = ctypes.c_int64

            @contextlib.contextmanager
            def _hook(output_dir, device_ids):
                import jax
                jax.devices()
                if device_ids:
                    ids = (ctypes.c_int64 * len(device_ids))(*device_ids)
                    rc = lib.axon_start_nrt_profile(ids, len(device_ids))
                else:
                    rc = lib.axon_start_nrt_profile(None, 0)
                if rc != 0:
                    raise RuntimeError(f"axon_start_nrt_profile rc={rc}")
                try:
                    yield
                finally:
                    n = lib.axon_stop_nrt_profile(str(output_dir).encode())
                    print(f"profile: {n} file(s) written to {output_dir}",
                          file=sys.stderr)

            hook = _hook
    except OSError:
        pass

    mod = types.ModuleType("antenv.axon_hooks")
    mod.get_axon_ntff_profile_hook = lambda: hook
    mod.set_axon_ntff_profile_hook = lambda h: None
    sys.modules["antenv.axon_hooks"] = mod


def _run_device(predicted_probs, true_winners, market_odds, trace=False):
    from concourse.bass_utils import run_bass_kernel_spmd

    if trace:
        _install_ntff_shim()
    nc = _get_program()
    in_maps = []
    for i in range(N_CORES):
        in_maps.append({
            "po": _shard_po(predicted_probs, true_winners, market_odds, i),
        })
    res = run_bass_kernel_spmd(nc, in_maps, list(range(N_CORES)), trace=trace)
    return res


def kernel(predicted_probs, true_winners, market_odds, _trace=False,
           _result_holder=None):
    res = _run_device(predicted_probs, true_winners, market_odds, trace=_trace)
    if _result_holder is not None:
        _result_holder.append(res)

    RELU = 0.0
    S_WO = 0.0
    NBS = 0.0
    NB_LAST = 0.0
    for i in range(N_CORES):
        a_s = res.results[i]["acc"].astype(np.float64)
        RELU += a_s[:, :NTT].sum()
        NBS += a_s[:, NTT:2 * NTT - 1].sum()
        NB_LAST += a_s[:, 2 * NTT - 1].sum()
        S_WO += a_s[:, 2 * NTT:].sum()
    # NBS = sum sign(best-1) over all but the last tile; the last tile
    # reports a direct count of best > 1 rows.
    B_REST = (ROWS_PP - ROW_TILES[-1]) * P * N_CORES
    num_bets = int(round((NBS + B_REST) / 2 + NB_LAST))

    if num_bets > 0:
        total_expected_profit = PAYOUT_SCALE * RELU
    else:
        total_expected_profit = -np.float64(
            np.mean(np.max(predicted_probs, axis=1))) * 0.1
    loss = -total_expected_profit / B
    batch_profit = PAYOUT_SCALE * S_WO - PAYOUT_SCALE * num_bets

    return (np.float32(loss), np.float32(batch_profit), np.int32(num_bets))


if __name__ == "__main__":
    rng = np.random.default_rng(0)
    probs = rng.random((B, T), dtype=np.float32)
    win = (rng.random((B, T)) > 0.8).astype(np.float32)
    odds = rng.random((B, T), dtype=np.float32) * 10.0
    odds[rng.random((B, 1))[:, 0] < 0.1] = 0.0
    out = kernel(probs, win, odds)
    print("kernel out:", out)
